# revision 12
# baseline (speedup 1.0000x reference)
"""Trainium2 Bass kernel for nn_BiholoModelFuncGENERALforHYMinv3.

Computation (per sample):
  x[18] -> 9 complex coords in 3 projective factors of 3
  bihom feature chain -> sec[729] (divided by kappa product)
  two towers: u1=(sec@W1+b1)^2 -> u2=(.@W2+b2)^2 -> u3=(.@W3+b3)^2
  out = Wfa*log(u3a) - Wfb*log(u3b), clipped to +-1e6

Distribution: pure data parallel over batch, 8 NeuronCores, 4096 samples
per core. Weights replicated.

On-chip layout: features/hidden units on the partition axis, batch on the
free axis. Weights then slice natively as matmul lhsT ([K,M] chunks of the
[in,out]-shaped DRAM arrays). The bihom chain runs on DVE with batch on
partitions (one broadcast-AP outer-product instruction per chain segment),
then PE-transposes to feature-on-partition for the tower matmuls.
Matmuls use the fp32r PE path (full fp32 operand bytes, 1 cycle/row at
moving-dim >= 256, vs 4 cycles/row for the 2-pass fp32 mode).
"""
import numpy as np

N_CORES = 8
B_FULL = 32768
B_CORE = B_FULL // N_CORES
N_TILE = 512          # moving-dim per tower pass (<= 512 for fp32 PSUM bank)
H = 1024              # hidden width
NSEC = 729
SEC_CHUNKS = [128, 128, 128, 128, 128, 89]   # 729 = 5*128 + 89
MM_DTYPE = "f32r"     # "f32r" | "f32" — PE matmul operand mode


def _brd(t_ap, free_dims, import_bass):
    """AP with t_ap's partition dim plus custom free [step,count] dims."""
    bass = import_bass
    return bass.AP(tensor=t_ap.tensor, offset=t_ap.offset,
                   ap=[list(t_ap.ap[0])] + [list(d) for d in free_dims])


def build_nc(b_core=B_CORE, n_tile=N_TILE, mm_dtype=MM_DTYPE, finalize=True):
    import concourse.bass as bass
    import concourse.tile as tile
    from concourse import mybir, bacc
    from concourse.masks import make_identity

    F32 = mybir.dt.float32
    F32R = mybir.dt.float32r
    AF = mybir.ActivationFunctionType
    ALU = mybir.AluOpType

    MMDT = F32R if mm_dtype == "f32r" else F32

    assert b_core % n_tile == 0 and n_tile % 128 == 0
    n_macro = b_core // n_tile
    n_sub = n_tile // 128

    nc = bacc.Bacc()
    x_d = nc.declare_dram_parameter("x", [b_core, 18], F32, isOutput=False)
    wd = {}
    for t in ("a", "b"):
        wd["W1" + t] = nc.declare_dram_parameter("W1" + t, [NSEC, H], MMDT, isOutput=False)
        wd["b1" + t] = nc.declare_dram_parameter("b1" + t, [H], F32, isOutput=False)
        wd["W2" + t] = nc.declare_dram_parameter("W2" + t, [H, H], MMDT, isOutput=False)
        wd["b2" + t] = nc.declare_dram_parameter("b2" + t, [H], F32, isOutput=False)
        wd["W3" + t] = nc.declare_dram_parameter("W3" + t, [H, 1], MMDT, isOutput=False)
        wd["b3" + t] = nc.declare_dram_parameter("b3" + t, [1], F32, isOutput=False)
        wd["Wf" + t] = nc.declare_dram_parameter("Wf" + t, [1, 1], F32, isOutput=False)
    out_d = nc.declare_dram_parameter("out", [b_core], F32, isOutput=True)

    with tile.TileContext(nc) as tc:
        import contextlib
        with contextlib.ExitStack() as ctx:
            consts = ctx.enter_context(tc.tile_pool(name="consts", bufs=1))
            xp = ctx.enter_context(tc.tile_pool(name="xp", bufs=4))
            ft = ctx.enter_context(tc.tile_pool(name="ft", bufs=2))
            secp = ctx.enter_context(tc.tile_pool(name="secp", bufs=4))
            ftp = ctx.enter_context(tc.tile_pool(name="ftp", bufs=1))
            qp = ctx.enter_context(tc.tile_pool(name="qp", bufs=1))
            q2p = ctx.enter_context(tc.tile_pool(name="q2p", bufs=1))
            ep = ctx.enter_context(tc.tile_pool(name="ep", bufs=2))
            psT = ctx.enter_context(tc.tile_pool(name="psT", bufs=2, space="PSUM"))
            psL = ctx.enter_context(tc.tile_pool(name="psL", bufs=3, space="PSUM"))
            psU = ctx.enter_context(tc.tile_pool(name="psU", bufs=2, space="PSUM"))

            # ---- constants / weights (resident) ----
            ident = consts.tile([128, 128], F32, tag="ident", name="ident")
            make_identity(nc, ident[:])

            W1 = {}; W2 = {}; W3 = {}; B1 = {}; B2 = {}; B3 = {}; WF = {}
            for t in ("a", "b"):
                # One DMA per weight tile: deps are tracked tile-granular, so
                # chunked loads would make every consumer matmul wait on all
                # chunk queues and blow the HW sync-wait slot budget.
                W1[t] = consts.tile([128, 6, H], MMDT, tag="W1" + t, name="W1" + t)
                nc.sync.dma_start(out=W1[t][:, 0:5, :],
                                  in_=wd["W1" + t][0:640, :].rearrange("(k p) h -> p k h", p=128))
                nc.sync.dma_start(out=W1[t][:89, 5, :], in_=wd["W1" + t][640:729, :])
                W2[t] = consts.tile([128, 8, H], MMDT, tag="W2" + t, name="W2" + t)
                nc.sync.dma_start(out=W2[t][:],
                                  in_=wd["W2" + t][:, :].rearrange("(k p) h -> p k h", p=128))
                W3[t] = consts.tile([128, 8, 1], MMDT, tag="W3" + t, name="W3" + t)
                nc.sync.dma_start(out=W3[t][:],
                                  in_=wd["W3" + t][:, :].rearrange("(k p) h -> p k h", p=128))
                B1[t] = consts.tile([128, 8], F32, tag="b1" + t, name="b1" + t)
                nc.sync.dma_start(out=B1[t][:], in_=wd["b1" + t].rearrange("(m p) -> p m", p=128))
                B2[t] = consts.tile([128, 8], F32, tag="b2" + t, name="b2" + t)
                nc.sync.dma_start(out=B2[t][:], in_=wd["b2" + t].rearrange("(m p) -> p m", p=128))
                B3[t] = consts.tile([1, 1], F32, tag="b3" + t, name="b3" + t)
                nc.sync.dma_start(out=B3[t][:], in_=wd["b3" + t].rearrange("(p o) -> p o", o=1))
                WF[t] = consts.tile([1, 1], F32, tag="Wf" + t, name="Wf" + t)
                nc.sync.dma_start(out=WF[t][:], in_=wd["Wf" + t][:, :])

            def feats_subtile(x_t):
                """Compute sec [128, 729] for one 128-sample subtile.
                x_t: [128, 18] slice (batch on partitions)."""
                xr = x_t[:, 0:9]
                xi = x_t[:, 9:18]

                # full 3x3 grids for all 3 factors: [128, 27], idx f*9+a*3+b
                XX = ft.tile([128, 27], F32, tag="XX", name="XX")
                nc.vector.tensor_mul(XX[:], _brd(xr, [[3, 3], [1, 3], [0, 3]], bass),
                                     _brd(xr, [[3, 3], [0, 3], [1, 3]], bass))
                XXYY = ft.tile([128, 27], F32, tag="XXYY", name="XXYY")
                nc.vector.tensor_mul(XXYY[:], _brd(xi, [[3, 3], [1, 3], [0, 3]], bass),
                                     _brd(xi, [[3, 3], [0, 3], [1, 3]], bass))
                nc.vector.tensor_add(XXYY[:], XXYY[:], XX[:])
                XY = ft.tile([128, 27], F32, tag="XY", name="XY")
                nc.vector.tensor_mul(XY[:], _brd(xr, [[3, 3], [1, 3], [0, 3]], bass),
                                     _brd(xi, [[3, 3], [0, 3], [1, 3]], bass))

                # r_all [128, 3, 6]: triu-gather cols {0,1,2,4,5,8} of each grid
                r_all = ft.tile([128, 3, 6], F32, tag="r_all", name="r_all")
                nc.vector.tensor_copy(_brd(r_all[:, 0, 0:3], [[6, 3], [1, 3]], bass),
                                      _brd(XXYY[:, 0:3], [[9, 3], [1, 3]], bass))
                nc.vector.tensor_copy(_brd(r_all[:, 0, 3:5], [[6, 3], [1, 2]], bass),
                                      _brd(XXYY[:, 4:6], [[9, 3], [1, 2]], bass))
                nc.vector.tensor_copy(_brd(r_all[:, 0, 5:6], [[6, 3], [1, 1]], bass),
                                      _brd(XXYY[:, 8:9], [[9, 3], [1, 1]], bass))
                # im_all [128, 3, 3]: XY[a,b]-XY[b,a] for (0,1),(0,2),(1,2)
                im_all = ft.tile([128, 3, 3], F32, tag="im_all", name="im_all")
                nc.vector.tensor_sub(_brd(im_all[:, 0, 0:2], [[3, 3], [1, 2]], bass),
                                     _brd(XY[:, 1:3], [[9, 3], [1, 2]], bass),
                                     _brd(XY[:, 3:7], [[9, 3], [3, 2]], bass))
                nc.vector.tensor_sub(_brd(im_all[:, 0, 2:3], [[3, 3], [1, 1]], bass),
                                     _brd(XY[:, 5:6], [[9, 3], [1, 1]], bass),
                                     _brd(XY[:, 7:8], [[9, 3], [1, 1]], bass))
                # kappa [128, 3] = diag sums; kprod, inv
                kap = ft.tile([128, 3], F32, tag="kap", name="kap")
                nc.vector.tensor_add(kap[:], _brd(XXYY[:, 0:1], [[9, 3]], bass),
                                     _brd(XXYY[:, 4:5], [[9, 3]], bass))
                nc.vector.tensor_add(kap[:], kap[:], _brd(XXYY[:, 8:9], [[9, 3]], bass))
                kp = ft.tile([128, 1], F32, tag="kp", name="kp")
                nc.vector.tensor_mul(kp[:], kap[:, 0:1], kap[:, 1:2])
                nc.vector.tensor_mul(kp[:], kp[:], kap[:, 2:3])
                inv = ft.tile([128, 1], F32, tag="inv", name="inv")
                nc.vector.reciprocal(inv[:], kp[:])

                i1n = ft.tile([128, 3], F32, tag="i1n", name="i1n")
                nc.vector.tensor_scalar_mul(i1n[:], im_all[:, 1, :], -1.0)
                rr2 = ft.tile([128, 6], F32, tag="rr2", name="rr2")
                nc.vector.tensor_mul(rr2[:], r_all[:, 2, :], _brd(inv[:, 0:1], [[0, 6]], bass))
                ii2 = ft.tile([128, 3], F32, tag="ii2", name="ii2")
                nc.vector.tensor_mul(ii2[:], im_all[:, 2, :], _brd(inv[:, 0:1], [[0, 3]], bass))

                r0 = r_all[:, 0, :]
                r1 = r_all[:, 1, :]
                i0 = im_all[:, 0, :]
                # level 2: R2 [128,45] = [r0 x r1 | i0 x i1n]; I2n [128,36] = [r0 x i1n | i0 x r1]
                R2 = ft.tile([128, 45], F32, tag="R2", name="R2")
                nc.vector.tensor_mul(R2[:, 0:36], _brd(r0, [[1, 6], [0, 6]], bass),
                                     _brd(r1, [[0, 6], [1, 6]], bass))
                nc.vector.tensor_mul(R2[:, 36:45], _brd(i0, [[1, 3], [0, 3]], bass),
                                     _brd(i1n[:, 0:3], [[0, 3], [1, 3]], bass))
                I2n = ft.tile([128, 36], F32, tag="I2n", name="I2n")
                nc.vector.tensor_mul(I2n[:, 0:18], _brd(r0, [[1, 6], [0, 3]], bass),
                                     _brd(i1n[:, 0:3], [[0, 6], [1, 3]], bass))
                nc.vector.tensor_mul(I2n[:, 18:36], _brd(i0, [[1, 3], [0, 6]], bass),
                                     _brd(r1, [[0, 3], [1, 6]], bass))
                # level 3 -> sec [128, 729]
                sec = secp.tile([128, NSEC], F32, tag="sec", name="sec")
                nc.vector.tensor_mul(sec[:, 0:270], _brd(R2[:, 0:45], [[1, 45], [0, 6]], bass),
                                     _brd(rr2[:, 0:6], [[0, 45], [1, 6]], bass))
                nc.vector.tensor_mul(sec[:, 270:378], _brd(I2n[:, 0:36], [[1, 36], [0, 3]], bass),
                                     _brd(ii2[:, 0:3], [[0, 36], [1, 3]], bass))
                nc.vector.tensor_mul(sec[:, 378:513], _brd(R2[:, 0:45], [[1, 45], [0, 3]], bass),
                                     _brd(ii2[:, 0:3], [[0, 45], [1, 3]], bass))
                nc.vector.tensor_mul(sec[:, 513:729], _brd(I2n[:, 0:36], [[1, 36], [0, 6]], bass),
                                     _brd(rr2[:, 0:6], [[0, 36], [1, 6]], bass))
                return sec

            for mt in range(n_macro):
                base = mt * n_tile
                # ---- features + transpose to featsT [128, 6, n_tile] ----
                featsT = ftp.tile([128, 6, n_tile], MMDT, tag="featsT", name="featsT")
                xm = xp.tile([128, n_sub, 18], F32, tag="xm", name="xm")
                nc.gpsimd.dma_start(
                    out=xm[:],
                    in_=x_d[base:base + n_tile, :].rearrange("(s p) c -> p s c", p=128))
                for s in range(n_sub):
                    sec = feats_subtile(xm[:, s, :])
                    for k in range(6):
                        kk = SEC_CHUNKS[k]
                        pT = psT.tile([128, 128], F32, tag="pT", name="pT")
                        nc.tensor.transpose(pT[:kk, :], sec[:, k * 128:k * 128 + kk],
                                            ident[:])
                        nc.vector.tensor_copy(featsT[:kk, k, s * 128:(s + 1) * 128],
                                              pT[:kk, :])

                # ---- towers ----
                lns = {}
                for t in ("a", "b"):
                    q1 = qp.tile([128, 8, n_tile], MMDT, tag="q1", name="q1")
                    for m in range(8):
                        ps = psL.tile([128, n_tile], F32, tag="mm", name="mm")
                        for k in range(6):
                            kk = SEC_CHUNKS[k]
                            nc.tensor.matmul(ps[:],
                                             W1[t][:kk, k, m * 128:(m + 1) * 128],
                                             featsT[:kk, k, :],
                                             start=(k == 0), stop=(k == 5))
                        nc.scalar.activation(out=q1[:, m, :], in_=ps[:], func=AF.Square,
                                             bias=B1[t][:, m:m + 1], scale=1.0)
                    q2 = q2p.tile([128, 8, n_tile], MMDT, tag="q2", name="q2")
                    for m in range(8):
                        ps = psL.tile([128, n_tile], F32, tag="mm", name="mm")
                        for k in range(8):
                            nc.tensor.matmul(ps[:],
                                             W2[t][:, k, m * 128:(m + 1) * 128],
                                             q1[:, k, :],
                                             start=(k == 0), stop=(k == 7))
                        nc.scalar.activation(out=q2[:, m, :], in_=ps[:], func=AF.Square,
                                             bias=B2[t][:, m:m + 1], scale=1.0)
                    u3 = psU.tile([1, n_tile], F32, tag="u3", name="u3")
                    for k in range(8):
                        nc.tensor.matmul(u3[:], W3[t][:, k, :],
                                         q2[:, k, :],
                                         start=(k == 0), stop=(k == 7))
                    ln_t = ep.tile([1, n_tile], F32, tag="ln" + t, name="ln" + t)
                    nc.scalar.activation(out=ln_t[:], in_=u3[:], func=AF.Square,
                                         bias=B3[t][0:1, 0:1], scale=1.0)
                    nc.scalar.activation(out=ln_t[:], in_=ln_t[:], func=AF.Ln)
                    lns[t] = ln_t

                # out = clip(Wfa*ln_a - Wfb*ln_b)
                nc.vector.tensor_scalar_mul(lns["b"][:], lns["b"][:], WF["b"][0:1, 0:1])
                o_t = ep.tile([1, n_tile], F32, tag="o", name="o")
                nc.vector.scalar_tensor_tensor(out=o_t[:], in0=lns["a"][:],
                                               scalar=WF["a"][0:1, 0:1], in1=lns["b"][:],
                                               op0=ALU.mult, op1=ALU.subtract)
                nc.vector.tensor_scalar(out=o_t[:], in0=o_t[:], scalar1=1.0e6,
                                        scalar2=-1.0e6, op0=ALU.min, op1=ALU.max)
                nc.sync.dma_start(out=out_d[base:base + n_tile], in_=o_t[:])

    if finalize:
        nc.finalize()   # Bacc pass pipeline: reg alloc, wait splitting, etc.
    return nc


def run(inputs, trace=False, b_core=B_CORE, n_tile=N_TILE, mm_dtype=MM_DTYPE,
        n_cores=N_CORES):
    """Shard inputs, run the SPMD kernel on n_cores, gather full output.
    Returns (out [B,1] fp32, BassKernelResults)."""
    from concourse import bass_utils

    nc = build_nc(b_core=b_core, n_tile=n_tile, mm_dtype=mm_dtype)
    x = np.ascontiguousarray(np.asarray(inputs["x"], dtype=np.float32))
    weights = {k: np.ascontiguousarray(np.asarray(v, dtype=np.float32))
               for k, v in inputs.items() if k != "x"}
    if mm_dtype == "f32r":
        # PE consumes these as fp32r; pre-round on host so DMA'd bits match
        # the hardware rounding semantics (round-to-nearest at 11 mantissa
        # bits, i.e. drop low 12 bits of the fp32 mantissa).
        for k in ("W1a", "W1b", "W2a", "W2b", "W3a", "W3b"):
            w = weights[k]
            u = w.view(np.uint32).astype(np.uint64)
            # round-half-even at bit 12
            lo = u & 0xFFF
            base = u & ~np.uint64(0xFFF)
            up = (lo > 0x800) | ((lo == 0x800) & (((u >> 12) & 1) == 1))
            u = base + (up.astype(np.uint64) << 12)
            weights[k] = u.astype(np.uint32).view(np.float32).astype(np.float32)
    in_maps = []
    for c in range(n_cores):
        m = {"x": x[c * b_core:(c + 1) * b_core]}
        m.update(weights)
        in_maps.append(m)
    res = bass_utils.run_bass_kernel_spmd(nc, in_maps, core_ids=list(range(n_cores)),
                                          trace=trace)
    out = np.concatenate([r["out"] for r in res.results], axis=0)
    return out.reshape(-1, 1).astype(np.float32), res


def kernel(**inputs) -> np.ndarray:
    out, _ = run(inputs, trace=False)
    return out
